# revision 57
# baseline (speedup 1.0000x reference)
"""Distributed KNN (analogy-based estimation) kernel for 8 TRN2 NeuronCores.

Strategy (scan-then-refine):
  - Shard the train set (N=65536) across 8 cores (8192 rows each); replicate
    the 2048 queries.  All tensors fit in SBUF, so HBM traffic is just the
    ~3MB/core input load.  No collectives - the merge happens on the host.
  - Device scan: fp8(e4m3) DoubleRow matmuls (K=256 in one instruction)
    compute s = scale * (x_hat . t) into PSUM f32.  The true distance's norm
    term only perturbs candidate ranking by O(1) while top-of-65536 gaps are
    O(10), so the cross term alone selects candidate cells safely.
  - Evacuation is the bottleneck (every PSUM value must cross a compute
    engine at ~1 elem/lane/cycle), so it is split three ways: 1 of 4 PSUM
    tiles goes straight to VectorE (reduce_max over 32-candidate cells);
    the other 3 go ScalarE (fused relu(s - T), made row-comparable by host
    query normalization) -> GpSimd (ADD-fold of tile halves) -> VectorE
    (small sum-reduce), producing a 256-cell statistic vector per (row,
    core) that is DMA'd out raw (top-k selection on host beats on-device
    max/max_index by ~22us of DVE critical path).
  - Host: top-16 cells per (row, core) by statistic, expand to ~4k candidate
    indices/row as contiguous 16-row blocks, coarse f32 distance pass
    narrows to 8 finalists, exact float64 pass ranks them with the
    reference's tie-breaking, then the label gather / faithful [B,k]->[k,B]
    reshape / integer-mean / one-hot epilogue in exact integer arithmetic.
"""

from contextlib import ExitStack

import numpy as np
import ml_dtypes

import concourse.bass as bass
import concourse.mybir as mybir
import concourse.tile as tile
from concourse import bacc
from concourse.bass_utils import run_bass_kernel_spmd

N_CORES = 8
B = 2048          # queries
N_TRAIN = 65536   # train rows
F = 256           # features
NSHARD = N_TRAIN // N_CORES   # 8192 train rows per core

Q_TILE = 128
N_QT = B // Q_TILE            # 16 query tiles
CHUNK_N = 512                 # matmul free dim == one PSUM bank (fp32)
N_CHUNKS = NSHARD // CHUNK_N  # 16
TILE_W = 1024                 # psum tile width (2 banks, 2 chunks)
N_PT = NSHARD // TILE_W       # 8 psum tiles per (q-tile, core)
CELLS_PER_TILE = 32           # scan cells per psum tile
N_CELLS = N_PT * CELLS_PER_TILE    # 256 cells per row per core
# Cell c of psum tile m covers candidate columns
#   m*1024 + 16c + [0..15]  and  m*1024 + 512 + 16c + [0..15]
# (32 candidates per cell; the split pairing comes from the GpSimd fold).
TOPC_HALF = 8                 # cells reported per (row, core, cmax-half)
N_HALVES = 2
TOPC = TOPC_HALF * N_HALVES   # 16 cells reported per (row, core)
# Scan statistic: queries are L2-normalized on the host, so s = x_hat . t has
# per-candidate std ~1 and global top-3 values ~3.5+.  Cells are ranked by
# sum(relu(s - RELU_T)) (or relu(max - RELU_T) on the DVE-direct tiles) —
# any cell holding a global top-3 value scores far above typical cells.  The
# top-8 is taken independently over each 128-cell half so a global top-3
# cell would need >= 8 stronger cells in its own half to be lost.
RELU_T = 2.5

_BF16 = mybir.dt.bfloat16
_F32 = mybir.dt.float32
_U32 = mybir.dt.uint32


DIRECT_MOD = 4   # psum tiles with m % DIRECT_MOD == 0 go DVE-direct
PE_ONLY = False  # benchmark probe: skip all PSUM evacuation
FP8 = True       # fp8(e4m3) DoubleRow matmul scan (K=256 per MM) vs bf16
FP8_SCALE = 32.0  # pre-scale on normalized queries so fp8 stays in range
NO_L2 = False    # benchmark probe: skip the top-8 max/max_index stage
NO_POOL = False  # benchmark probe: DVE reduces ACT output (no GpSimd fold)


def _build(loop_reps=None):
    in_dt = mybir.dt.float8e4 if FP8 else _BF16
    nc = bacc.Bacc("TRN2", target_bir_lowering=False, debug=False)
    xT = nc.dram_tensor("xT", [F, B], in_dt, kind="ExternalInput")
    tT = nc.dram_tensor("tT", [F, NSHARD], in_dt, kind="ExternalInput")
    out_cm = nc.dram_tensor("cmax_out", [B, N_CELLS], _F32, kind="ExternalOutput")

    with tile.TileContext(nc) as tc, ExitStack() as ctx:
        const = ctx.enter_context(tc.tile_pool(name="const", bufs=1))
        psums = ctx.enter_context(tc.tile_pool(name="ps", bufs=4, space="PSUM"))
        cmaxp = ctx.enter_context(tc.tile_pool(name="cmax", bufs=2))
        stagep = ctx.enter_context(tc.tile_pool(name="stage", bufs=3))
        gpsp = ctx.enter_context(tc.tile_pool(name="gps", bufs=3))

        # Load order matters: the first PSUM wave needs xT q-tile 0 and tT
        # chunks 0..7, so stage those DMAs first.
        xT_sb = [[None] * N_QT for _ in range(2)]
        tT_sb = [[None] * N_CHUNKS for _ in range(2)]

        if FP8:
            # One [128, 2*W] tile per q-tile/chunk: both 128-feature halves
            # concatenated along free, for DoubleRow's [p, 2, w] operand AP.
            def load_x(q):
                xs = const.tile([128, 2 * Q_TILE], mybir.dt.float8e4,
                                tag=f"xdr_{q}", name=f"xdr_{q}")
                for f in range(2):
                    nc.sync.dma_start(
                        xs[:, f * Q_TILE:(f + 1) * Q_TILE],
                        xT[f * 128:(f + 1) * 128, q * Q_TILE:(q + 1) * Q_TILE],
                    )
                xT_sb[0][q] = xs

            def load_t(c):
                ts_ = const.tile([128, 2 * CHUNK_N], mybir.dt.float8e4,
                                 tag=f"tdr_{c}", name=f"tdr_{c}")
                for f in range(2):
                    nc.sync.dma_start(
                        ts_[:, f * CHUNK_N:(f + 1) * CHUNK_N],
                        tT[f * 128:(f + 1) * 128, c * CHUNK_N:(c + 1) * CHUNK_N],
                    )
                tT_sb[0][c] = ts_
        else:
            def load_x(q):
                for f in range(2):
                    xs = const.tile([128, Q_TILE], _BF16, tag=f"xT{f}_{q}",
                                    name=f"xT{f}_{q}")
                    nc.sync.dma_start(
                        xs[:], xT[f * 128:(f + 1) * 128, q * Q_TILE:(q + 1) * Q_TILE]
                    )
                    xT_sb[f][q] = xs

            def load_t(c):
                for f in range(2):
                    ts_ = const.tile([128, CHUNK_N], _BF16, tag=f"tT{f}_{c}",
                                     name=f"tT{f}_{c}")
                    nc.sync.dma_start(
                        ts_[:],
                        tT[f * 128:(f + 1) * 128, c * CHUNK_N:(c + 1) * CHUNK_N]
                    )
                    tT_sb[f][c] = ts_

        load_x(0)
        for c in range(N_CHUNKS):
            load_t(c)
        for q in range(1, N_QT):
            load_x(q)

        neg_t = const.tile([128, 1], _F32, name="neg_t")
        nc.vector.memset(neg_t[:], -RELU_T * (FP8_SCALE if FP8 else 1.0))

        CPW = TILE_W // CHUNK_N  # chunks per psum tile
        TILES_PER_WAVE = 2       # tiles per accumulation wave
        WAVES = N_PT // TILES_PER_WAVE  # 4 waves per q-tile

        def compute():
            _compute(nc, tc, xT_sb, tT_sb, neg_t, cmaxp, psums, stagep,
                     gpsp, out_cm, CPW, TILES_PER_WAVE, WAVES)

        if loop_reps is not None:
            with tc.For_i(0, loop_reps, 1):
                compute()
        else:
            compute()
    nc.compile()
    return nc


def _compute(nc, tc, xT_sb, tT_sb, neg_t, cmaxp, psums, stagep, gpsp,
             out_cm, CPW, TILES_PER_WAVE, WAVES):
        for q in range(N_QT):
            qs = slice(q * Q_TILE, (q + 1) * Q_TILE)
            cmax = cmaxp.tile([128, N_CELLS], _F32, name=f"cmax_{q}")
            for w in range(WAVES):
                pss = [
                    psums.tile([128, TILE_W], _F32, tag="ps", name=f"ps_{q}_{w}_{j}")
                    for j in range(TILES_PER_WAVE)
                ]
                if FP8:
                    for j in range(TILES_PER_WAVE):
                        for hh in range(CPW):
                            c = (w * TILES_PER_WAVE + j) * CPW + hh
                            nc.tensor.matmul(
                                pss[j][:, hh * CHUNK_N:(hh + 1) * CHUNK_N],
                                xT_sb[0][q][:].rearrange(
                                    "p (i m) -> p i m", i=2
                                ),
                                tT_sb[0][c][:].rearrange(
                                    "p (i n) -> p i n", i=2
                                ),
                                start=True,
                                stop=True,
                                perf_mode=mybir.MatmulPerfMode.DoubleRow,
                            )
                else:
                    for f in range(2):  # contraction tiles of 128 features
                        for j in range(TILES_PER_WAVE):
                            for hh in range(CPW):
                                c = (w * TILES_PER_WAVE + j) * CPW + hh
                                nc.tensor.matmul(
                                    pss[j][:, hh * CHUNK_N:(hh + 1) * CHUNK_N],
                                    xT_sb[f][q][:],
                                    tT_sb[f][c][:],
                                    start=(f == 0),
                                    stop=(f == 1),
                                )
                for j in range(TILES_PER_WAVE):
                    m = w * TILES_PER_WAVE + j
                    cm_out = cmax[:, m * CELLS_PER_TILE:(m + 1) * CELLS_PER_TILE]
                    if PE_ONLY:
                        if m == 0:
                            nc.vector.memset(cmax[:], 0.0)
                        continue
                    if DIRECT_MOD > 0 and m % DIRECT_MOD == 0:
                        # DVE evacuates+reduces this tile straight from PSUM
                        # (max over 32 contiguous candidates per cell), then
                        # ScalarE maps it onto the relu(.-T) scale.
                        # Raw max out; the host subtracts RELU_T from these
                        # columns before selection (saves an ACT op here).
                        nc.vector.tensor_reduce(
                            out=cm_out,
                            in_=pss[j][:].rearrange("p (c e) -> p c e", e=32),
                            axis=mybir.AxisListType.X,
                            op=mybir.AluOpType.max,
                        )
                    else:
                        # Offload evacuation: ScalarE applies relu(s - T) on
                        # the way PSUM->SBUF, GpSimd ADD-folds the two
                        # 512-halves, DVE sum-reduces 16-wide cells.
                        st = stagep.tile([128, TILE_W], _F32, tag="st",
                                         name=f"st_{q}_{m}")
                        nc.scalar.activation(
                            st[:], pss[j][:],
                            mybir.ActivationFunctionType.Relu,
                            bias=neg_t[:],
                        )
                        if NO_POOL:
                            nc.vector.tensor_reduce(
                                out=cm_out,
                                in_=st[:].rearrange(
                                    "p (two c e) -> p c two e", two=2, e=16
                                ),
                                axis=mybir.AxisListType.XY,
                                op=mybir.AluOpType.add,
                            )
                        else:
                            gp = gpsp.tile([128, TILE_W // 2], _F32, tag="gp",
                                           name=f"gp_{q}_{m}")
                            nc.gpsimd.tensor_add(
                                gp[:], st[:, 0:TILE_W // 2],
                                st[:, TILE_W // 2:TILE_W]
                            )
                            nc.vector.tensor_reduce(
                                out=cm_out,
                                in_=gp[:].rearrange("p (c e) -> p c e", e=16),
                                axis=mybir.AxisListType.X,
                                op=mybir.AluOpType.add,
                            )
            nc.sync.dma_start(out_cm[qs, :], cmax[:])


def _cells_to_blocks(cid):
    """Map per-(core,row) cell ids -> the two 16-row train blocks each covers.

    cid: int64 array of cell ids in [0, N_CELLS), AFTER the half-offset fix.
    Returns (blk0, blk1) within-shard block indices (block = 16 train rows).
    DVE-direct tiles (m % DIRECT_MOD == 0) use 32 contiguous candidates;
    offloaded tiles pair candidates {16c, 16c+512} (the GpSimd fold).
    """
    m = cid // CELLS_PER_TILE
    c = cid % CELLS_PER_TILE
    direct = (m % DIRECT_MOD == 0) if DIRECT_MOD > 0 else np.zeros_like(m, bool)
    blk0 = m * (TILE_W // 16) + np.where(direct, 2 * c, c)
    blk1 = blk0 + np.where(direct, 1, TILE_W // 32)
    return blk0, blk1


def _host_adjust(cm):
    """Direct tiles report raw cell max; put them on the relu(.-T) scale."""
    shift = RELU_T * (FP8_SCALE if FP8 else 1.0)
    m = np.arange(N_CELLS) // CELLS_PER_TILE
    direct = (m % DIRECT_MOD == 0) if DIRECT_MOD > 0 else np.zeros_like(m, bool)
    cm[..., direct] -= shift
    return cm


_CACHE = {}


def _run_device(x_input, train_inputs, trace=False, **kw):
    if "nc" not in _CACHE:
        _CACHE["nc"] = _build()
    nc = _CACHE["nc"]
    x = np.asarray(x_input, np.float32)
    # Row-normalize queries so the global RELU_T threshold is calibrated.
    xh = x / (np.linalg.norm(x, axis=1, keepdims=True) + 1e-30)
    if FP8:
        in_np_dt = ml_dtypes.float8_e4m3
        xh = xh * FP8_SCALE
    else:
        in_np_dt = ml_dtypes.bfloat16
    xT = np.ascontiguousarray(xh.T).astype(in_np_dt)
    in_maps = []
    for s in range(N_CORES):
        shard = np.asarray(train_inputs[s * NSHARD:(s + 1) * NSHARD], np.float32)
        tTs = np.ascontiguousarray(shard.T).astype(in_np_dt)
        in_maps.append({"xT": xT, "tT": tTs})
    return run_bass_kernel_spmd(
        nc, in_maps, core_ids=list(range(N_CORES)), trace=trace, **kw
    )


def kernel(x_input, train_inputs, features, train_labels, num_k, num_labels):
    x = np.asarray(x_input, dtype=np.float32)
    train = np.asarray(train_inputs, dtype=np.float32)
    feats = np.asarray(features, dtype=np.float32)
    labels = np.asarray(train_labels)
    k = int(num_k)
    L = int(num_labels)

    res = _run_device(x, train)
    cm = np.stack(
        [np.asarray(res.results[s]["cmax_out"]) for s in range(N_CORES)], axis=0
    )  # [cores, B, N_CELLS] f32 cell statistics
    cm = _host_adjust(cm)

    # Host-side selection: top-TOPC cells per (core, row) by statistic.
    flat = cm.reshape(-1, N_CELLS)
    part = np.argpartition(-flat, TOPC - 1, axis=1)[:, :TOPC]
    cid = part.reshape(N_CORES, B, TOPC).astype(np.int64)

    # Expand top cells to candidate BLOCKS of 16 contiguous train rows.
    blk0, blk1 = _cells_to_blocks(cid)
    blk = np.stack([blk0, blk1], axis=-1)             # [cores, B, TOPC, 2]
    blk = blk + (np.arange(N_CORES, dtype=np.int64) * (NSHARD // 16))[
        :, None, None, None
    ]
    blk = blk.transpose(1, 0, 2, 3).reshape(B, -1)    # [B, cores*TOPC*2=256]
    blk = np.sort(blk, axis=1)
    NBLK = blk.shape[1]
    dupb = np.zeros(blk.shape, dtype=bool)
    dupb[:, 1:] = blk[:, 1:] == blk[:, :-1]

    # Refinement: coarse f32 pass narrows ~4k candidates/row to 8, then an
    # exact float64 pass ranks those with the reference's tie-breaking.
    w = feats[None, :] * train
    right32 = np.einsum("nf,nf->n", w, w, dtype=np.float32)
    left32 = np.einsum("bf,bf->b", x, x, dtype=np.float32)
    w64 = w.astype(np.float64)
    x64 = x.astype(np.float64)
    left64 = np.einsum("bf,bf->b", x64, x64)

    train_blocks = train.reshape(N_TRAIN // 16, 16 * F)
    NARROW = 8
    topk_idx = np.empty((B, k), dtype=np.int64)
    CH = 128
    gbuf = np.empty((CH * NBLK, 16 * F), dtype=np.float32)
    for r0 in range(0, B, CH):
        r1 = min(B, r0 + CH)
        bi = blk[r0:r1]                                # [rows, NBLK]
        ci = (bi[:, :, None] * 16 + np.arange(16)).reshape(r1 - r0, -1)
        np.take(train_blocks, bi.ravel(), axis=0, out=gbuf)
        tcand = gbuf.reshape(r1 - r0, NBLK * 16, F)    # [rows, nc, F]
        cross = np.matmul(tcand, x[r0:r1][:, :, None])[..., 0]
        d32 = np.sqrt(left32[r0:r1, None] + right32[ci]) - 2.0 * cross
        d32.reshape(r1 - r0, NBLK, 16)[dupb[r0:r1]] = np.inf
        part = np.argpartition(d32, NARROW, axis=1)[:, :NARROW]
        ci8 = np.take_along_axis(ci, part, axis=1)     # [rows, 8] distinct
        ci8.sort(axis=1)
        # exact f64 distances for the 8 finalists
        t8 = train[ci8].astype(np.float64)
        cross8 = np.matmul(t8, x64[r0:r1][:, :, None])[..., 0]
        w8 = w64[ci8]
        r8 = np.einsum("bkf,bkf->bk", w8, w8)
        d8 = np.sqrt(left64[r0:r1, None] + r8) - 2.0 * cross8
        dup8 = np.zeros(ci8.shape, dtype=bool)
        dup8[:, 1:] = ci8[:, 1:] == ci8[:, :-1]
        d8[dup8] = np.inf
        order = np.argsort(d8, axis=1, kind="stable")[:, :k]
        topk_idx[r0:r1] = np.take_along_axis(ci8, order, axis=1)

    lab = labels[topk_idx]               # [B, k] (int64)
    lab_kb = lab.reshape(k, B)           # faithful [B,k] -> [k,B] reshape
    outputs = lab_kb.sum(axis=0) // k
    out = np.zeros((B, L), dtype=np.float32)
    out[np.arange(B), outputs] = 1.0
    return out


# revision 60
# speedup vs baseline: 1.1551x; 1.1551x over previous
"""Distributed KNN (analogy-based estimation) kernel for 8 TRN2 NeuronCores.

Strategy (scan-then-refine):
  - Shard the train set (N=65536) across 8 cores (8192 rows each); replicate
    the 2048 queries.  All tensors fit in SBUF, so HBM traffic is just the
    ~3MB/core input load.  No collectives - the merge happens on the host.
  - Device scan: fp8(e4m3) DoubleRow matmuls (K=256 in one instruction)
    compute s = scale * (x_hat . t) into PSUM f32.  The true distance's norm
    term only perturbs candidate ranking by O(1) while top-of-65536 gaps are
    O(10), so the cross term alone selects candidate cells safely.
  - Evacuation is the bottleneck (every PSUM value must cross a compute
    engine at ~1 elem/lane/cycle), so it is split three ways: 1 of 4 PSUM
    tiles goes straight to VectorE (reduce_max over 32-candidate cells);
    the other 3 go ScalarE (fused relu(s - T), made row-comparable by host
    query normalization) -> GpSimd (ADD-fold of tile halves) -> VectorE
    (small sum-reduce), producing a 256-cell statistic vector per (row,
    core) that is DMA'd out raw (top-k selection on host beats on-device
    max/max_index by ~22us of DVE critical path).
  - Host: top-16 cells per (row, core) by statistic, expand to ~4k candidate
    indices/row as contiguous 16-row blocks, coarse f32 distance pass
    narrows to 8 finalists, exact float64 pass ranks them with the
    reference's tie-breaking, then the label gather / faithful [B,k]->[k,B]
    reshape / integer-mean / one-hot epilogue in exact integer arithmetic.
"""

from contextlib import ExitStack

import numpy as np
import ml_dtypes

import concourse.bass as bass
import concourse.mybir as mybir
import concourse.tile as tile
from concourse import bacc
from concourse.bass_utils import run_bass_kernel_spmd

N_CORES = 8
B = 2048          # queries
N_TRAIN = 65536   # train rows
F = 256           # features
NSHARD = N_TRAIN // N_CORES   # 8192 train rows per core

Q_TILE = 128
N_QT = B // Q_TILE            # 16 query tiles
CHUNK_N = 512                 # matmul free dim == one PSUM bank (fp32)
N_CHUNKS = NSHARD // CHUNK_N  # 16
TILE_W = 1024                 # psum tile width (2 banks, 2 chunks)
N_PT = NSHARD // TILE_W       # 8 psum tiles per (q-tile, core)
CELLS_PER_TILE = 32           # scan cells per psum tile
N_CELLS = N_PT * CELLS_PER_TILE    # 256 cells per row per core
# Cell c of psum tile m covers candidate columns
#   m*1024 + 16c + [0..15]  and  m*1024 + 512 + 16c + [0..15]
# (32 candidates per cell; the split pairing comes from the GpSimd fold).
TOPC_HALF = 8                 # cells reported per (row, core, cmax-half)
N_HALVES = 2
TOPC = TOPC_HALF * N_HALVES   # 16 cells reported per (row, core)
# Scan statistic: queries are L2-normalized on the host, so s = x_hat . t has
# per-candidate std ~1 and global top-3 values ~3.5+.  Cells are ranked by
# sum(relu(s - RELU_T)) (or relu(max - RELU_T) on the DVE-direct tiles) —
# any cell holding a global top-3 value scores far above typical cells.  The
# top-8 is taken independently over each 128-cell half so a global top-3
# cell would need >= 8 stronger cells in its own half to be lost.
RELU_T = 2.5

_BF16 = mybir.dt.bfloat16
_F32 = mybir.dt.float32
_U32 = mybir.dt.uint32


DIRECT_MOD = 4   # psum tiles with m % DIRECT_MOD == 0 go DVE-direct
PE_ONLY = False  # benchmark probe: skip all PSUM evacuation
FP8 = True       # fp8(e4m3) DoubleRow matmul scan (K=256 per MM) vs bf16
FP8_SCALE = 32.0  # pre-scale on normalized queries so fp8 stays in range
NO_L2 = False    # benchmark probe: skip the top-8 max/max_index stage
NO_POOL = False  # benchmark probe: DVE reduces ACT output (no GpSimd fold)


def _build(loop_reps=None):
    in_dt = mybir.dt.float8e4 if FP8 else _BF16
    nc = bacc.Bacc("TRN2", target_bir_lowering=False, debug=False)
    xT = nc.dram_tensor("xT", [F, B], in_dt, kind="ExternalInput")
    tT = nc.dram_tensor("tT", [F, NSHARD], in_dt, kind="ExternalInput")
    out_cm = nc.dram_tensor("cmax_out", [B, N_CELLS], _F32, kind="ExternalOutput")

    with tile.TileContext(nc) as tc, ExitStack() as ctx:
        const = ctx.enter_context(tc.tile_pool(name="const", bufs=1))
        psums = ctx.enter_context(tc.tile_pool(name="ps", bufs=4, space="PSUM"))
        cmaxp = ctx.enter_context(tc.tile_pool(name="cmax", bufs=2))
        stagep = ctx.enter_context(tc.tile_pool(name="stage", bufs=3))
        gpsp = ctx.enter_context(tc.tile_pool(name="gps", bufs=3))

        # Load order matters: the first PSUM wave needs xT q-tile 0 and tT
        # chunks 0..7, so stage those DMAs first.
        xT_sb = [[None] * N_QT for _ in range(2)]
        tT_sb = [[None] * N_CHUNKS for _ in range(2)]

        if FP8:
            # One [128, 2*W] tile per q-tile/chunk: both 128-feature halves
            # concatenated along free, for DoubleRow's [p, 2, w] operand AP.
            def load_x(q):
                xs = const.tile([128, 2 * Q_TILE], mybir.dt.float8e4,
                                tag=f"xdr_{q}", name=f"xdr_{q}")
                for f in range(2):
                    nc.sync.dma_start(
                        xs[:, f * Q_TILE:(f + 1) * Q_TILE],
                        xT[f * 128:(f + 1) * 128, q * Q_TILE:(q + 1) * Q_TILE],
                    )
                xT_sb[0][q] = xs

            def load_t(c):
                ts_ = const.tile([128, 2 * CHUNK_N], mybir.dt.float8e4,
                                 tag=f"tdr_{c}", name=f"tdr_{c}")
                for f in range(2):
                    nc.sync.dma_start(
                        ts_[:, f * CHUNK_N:(f + 1) * CHUNK_N],
                        tT[f * 128:(f + 1) * 128, c * CHUNK_N:(c + 1) * CHUNK_N],
                    )
                tT_sb[0][c] = ts_
        else:
            def load_x(q):
                for f in range(2):
                    xs = const.tile([128, Q_TILE], _BF16, tag=f"xT{f}_{q}",
                                    name=f"xT{f}_{q}")
                    nc.sync.dma_start(
                        xs[:], xT[f * 128:(f + 1) * 128, q * Q_TILE:(q + 1) * Q_TILE]
                    )
                    xT_sb[f][q] = xs

            def load_t(c):
                for f in range(2):
                    ts_ = const.tile([128, CHUNK_N], _BF16, tag=f"tT{f}_{c}",
                                     name=f"tT{f}_{c}")
                    nc.sync.dma_start(
                        ts_[:],
                        tT[f * 128:(f + 1) * 128, c * CHUNK_N:(c + 1) * CHUNK_N]
                    )
                    tT_sb[f][c] = ts_

        load_x(0)
        for c in range(N_CHUNKS):
            load_t(c)
        for q in range(1, N_QT):
            load_x(q)

        neg_t = const.tile([128, 1], _F32, name="neg_t")
        nc.vector.memset(neg_t[:], -RELU_T * (FP8_SCALE if FP8 else 1.0))

        CPW = TILE_W // CHUNK_N  # chunks per psum tile
        TILES_PER_WAVE = 2       # tiles per accumulation wave
        WAVES = N_PT // TILES_PER_WAVE  # 4 waves per q-tile

        def compute():
            _compute(nc, tc, xT_sb, tT_sb, neg_t, cmaxp, psums, stagep,
                     gpsp, out_cm, CPW, TILES_PER_WAVE, WAVES)

        if loop_reps is not None:
            with tc.For_i(0, loop_reps, 1):
                compute()
        else:
            compute()
    nc.compile()
    return nc


def _compute(nc, tc, xT_sb, tT_sb, neg_t, cmaxp, psums, stagep, gpsp,
             out_cm, CPW, TILES_PER_WAVE, WAVES):
        cmaxes = {}

        def emit_wave(q, w, cmax):
                pss = [
                    psums.tile([128, TILE_W], _F32, tag="ps", name=f"ps_{q}_{w}_{j}")
                    for j in range(TILES_PER_WAVE)
                ]
                if FP8:
                    for j in range(TILES_PER_WAVE):
                        for hh in range(CPW):
                            c = (w * TILES_PER_WAVE + j) * CPW + hh
                            nc.tensor.matmul(
                                pss[j][:, hh * CHUNK_N:(hh + 1) * CHUNK_N],
                                xT_sb[0][q][:].rearrange(
                                    "p (i m) -> p i m", i=2
                                ),
                                tT_sb[0][c][:].rearrange(
                                    "p (i n) -> p i n", i=2
                                ),
                                start=True,
                                stop=True,
                                perf_mode=mybir.MatmulPerfMode.DoubleRow,
                            )
                else:
                    for f in range(2):  # contraction tiles of 128 features
                        for j in range(TILES_PER_WAVE):
                            for hh in range(CPW):
                                c = (w * TILES_PER_WAVE + j) * CPW + hh
                                nc.tensor.matmul(
                                    pss[j][:, hh * CHUNK_N:(hh + 1) * CHUNK_N],
                                    xT_sb[f][q][:],
                                    tT_sb[f][c][:],
                                    start=(f == 0),
                                    stop=(f == 1),
                                )
                for j in range(TILES_PER_WAVE):
                    m = w * TILES_PER_WAVE + j
                    cm_out = cmax[:, m * CELLS_PER_TILE:(m + 1) * CELLS_PER_TILE]
                    if PE_ONLY:
                        if m == 0:
                            nc.vector.memset(cmax[:], 0.0)
                        continue
                    if DIRECT_MOD > 0 and m % DIRECT_MOD == 0:
                        # DVE evacuates+reduces this tile straight from PSUM
                        # (max over 32 contiguous candidates per cell), then
                        # ScalarE maps it onto the relu(.-T) scale.
                        # Raw max out; the host subtracts RELU_T from these
                        # columns before selection (saves an ACT op here).
                        nc.vector.tensor_reduce(
                            out=cm_out,
                            in_=pss[j][:].rearrange("p (c e) -> p c e", e=32),
                            axis=mybir.AxisListType.X,
                            op=mybir.AluOpType.max,
                        )
                    else:
                        # Offload evacuation: ScalarE applies relu(s - T) on
                        # the way PSUM->SBUF, GpSimd ADD-folds the two
                        # 512-halves, DVE sum-reduces 16-wide cells.
                        st = stagep.tile([128, TILE_W], _BF16, tag="st",
                                         name=f"st_{q}_{m}")
                        nc.scalar.activation(
                            st[:], pss[j][:],
                            mybir.ActivationFunctionType.Relu,
                            bias=neg_t[:],
                        )
                        if NO_POOL:
                            nc.vector.tensor_reduce(
                                out=cm_out,
                                in_=st[:].rearrange(
                                    "p (two c e) -> p c two e", two=2, e=16
                                ),
                                axis=mybir.AxisListType.XY,
                                op=mybir.AluOpType.add,
                            )
                        else:
                            gp = gpsp.tile([128, TILE_W // 2], _BF16, tag="gp",
                                           name=f"gp_{q}_{m}")
                            nc.gpsimd.tensor_add(
                                gp[:], st[:, 0:TILE_W // 2],
                                st[:, TILE_W // 2:TILE_W]
                            )
                            nc.vector.tensor_reduce(
                                out=cm_out,
                                in_=gp[:].rearrange("p (c e) -> p c e", e=16),
                                axis=mybir.AxisListType.X,
                                op=mybir.AluOpType.add,
                            )
        for q in range(N_QT):
            cmax = cmaxp.tile([128, N_CELLS], _F32, name=f"cmax_{q}")
            for w in range(WAVES):
                emit_wave(q, w, cmax)
            qs = slice(q * Q_TILE, (q + 1) * Q_TILE)
            nc.sync.dma_start(out_cm[qs, :], cmax[:])


def _cells_to_blocks(cid):
    """Map per-(core,row) cell ids -> the two 16-row train blocks each covers.

    cid: int64 array of cell ids in [0, N_CELLS), AFTER the half-offset fix.
    Returns (blk0, blk1) within-shard block indices (block = 16 train rows).
    DVE-direct tiles (m % DIRECT_MOD == 0) use 32 contiguous candidates;
    offloaded tiles pair candidates {16c, 16c+512} (the GpSimd fold).
    """
    m = cid // CELLS_PER_TILE
    c = cid % CELLS_PER_TILE
    direct = (m % DIRECT_MOD == 0) if DIRECT_MOD > 0 else np.zeros_like(m, bool)
    blk0 = m * (TILE_W // 16) + np.where(direct, 2 * c, c)
    blk1 = blk0 + np.where(direct, 1, TILE_W // 32)
    return blk0, blk1


def _host_adjust(cm):
    """Direct tiles report raw cell max; put them on the relu(.-T) scale."""
    shift = RELU_T * (FP8_SCALE if FP8 else 1.0)
    m = np.arange(N_CELLS) // CELLS_PER_TILE
    direct = (m % DIRECT_MOD == 0) if DIRECT_MOD > 0 else np.zeros_like(m, bool)
    cm[..., direct] -= shift
    return cm


_CACHE = {}


def _run_device(x_input, train_inputs, trace=False, **kw):
    if "nc" not in _CACHE:
        _CACHE["nc"] = _build()
    nc = _CACHE["nc"]
    x = np.asarray(x_input, np.float32)
    # Row-normalize queries so the global RELU_T threshold is calibrated.
    xh = x / (np.linalg.norm(x, axis=1, keepdims=True) + 1e-30)
    if FP8:
        in_np_dt = ml_dtypes.float8_e4m3
        xh = xh * FP8_SCALE
    else:
        in_np_dt = ml_dtypes.bfloat16
    xT = np.ascontiguousarray(xh.T).astype(in_np_dt)
    in_maps = []
    for s in range(N_CORES):
        shard = np.asarray(train_inputs[s * NSHARD:(s + 1) * NSHARD], np.float32)
        tTs = np.ascontiguousarray(shard.T).astype(in_np_dt)
        in_maps.append({"xT": xT, "tT": tTs})
    return run_bass_kernel_spmd(
        nc, in_maps, core_ids=list(range(N_CORES)), trace=trace, **kw
    )


def kernel(x_input, train_inputs, features, train_labels, num_k, num_labels):
    x = np.asarray(x_input, dtype=np.float32)
    train = np.asarray(train_inputs, dtype=np.float32)
    feats = np.asarray(features, dtype=np.float32)
    labels = np.asarray(train_labels)
    k = int(num_k)
    L = int(num_labels)

    res = _run_device(x, train)
    cm = np.stack(
        [np.asarray(res.results[s]["cmax_out"]) for s in range(N_CORES)], axis=0
    )  # [cores, B, N_CELLS] f32 cell statistics
    cm = _host_adjust(cm)

    # Host-side selection: top-TOPC cells per (core, row) by statistic.
    flat = cm.reshape(-1, N_CELLS)
    part = np.argpartition(-flat, TOPC - 1, axis=1)[:, :TOPC]
    cid = part.reshape(N_CORES, B, TOPC).astype(np.int64)

    # Expand top cells to candidate BLOCKS of 16 contiguous train rows.
    blk0, blk1 = _cells_to_blocks(cid)
    blk = np.stack([blk0, blk1], axis=-1)             # [cores, B, TOPC, 2]
    blk = blk + (np.arange(N_CORES, dtype=np.int64) * (NSHARD // 16))[
        :, None, None, None
    ]
    blk = blk.transpose(1, 0, 2, 3).reshape(B, -1)    # [B, cores*TOPC*2=256]
    blk = np.sort(blk, axis=1)
    NBLK = blk.shape[1]
    dupb = np.zeros(blk.shape, dtype=bool)
    dupb[:, 1:] = blk[:, 1:] == blk[:, :-1]

    # Refinement: coarse f32 pass narrows ~4k candidates/row to 8, then an
    # exact float64 pass ranks those with the reference's tie-breaking.
    w = feats[None, :] * train
    right32 = np.einsum("nf,nf->n", w, w, dtype=np.float32)
    left32 = np.einsum("bf,bf->b", x, x, dtype=np.float32)
    w64 = w.astype(np.float64)
    x64 = x.astype(np.float64)
    left64 = np.einsum("bf,bf->b", x64, x64)

    train_blocks = train.reshape(N_TRAIN // 16, 16 * F)
    NARROW = 8
    topk_idx = np.empty((B, k), dtype=np.int64)
    CH = 128
    gbuf = np.empty((CH * NBLK, 16 * F), dtype=np.float32)
    for r0 in range(0, B, CH):
        r1 = min(B, r0 + CH)
        bi = blk[r0:r1]                                # [rows, NBLK]
        ci = (bi[:, :, None] * 16 + np.arange(16)).reshape(r1 - r0, -1)
        np.take(train_blocks, bi.ravel(), axis=0, out=gbuf)
        tcand = gbuf.reshape(r1 - r0, NBLK * 16, F)    # [rows, nc, F]
        cross = np.matmul(tcand, x[r0:r1][:, :, None])[..., 0]
        d32 = np.sqrt(left32[r0:r1, None] + right32[ci]) - 2.0 * cross
        d32.reshape(r1 - r0, NBLK, 16)[dupb[r0:r1]] = np.inf
        part = np.argpartition(d32, NARROW, axis=1)[:, :NARROW]
        ci8 = np.take_along_axis(ci, part, axis=1)     # [rows, 8] distinct
        ci8.sort(axis=1)
        # exact f64 distances for the 8 finalists
        t8 = train[ci8].astype(np.float64)
        cross8 = np.matmul(t8, x64[r0:r1][:, :, None])[..., 0]
        w8 = w64[ci8]
        r8 = np.einsum("bkf,bkf->bk", w8, w8)
        d8 = np.sqrt(left64[r0:r1, None] + r8) - 2.0 * cross8
        dup8 = np.zeros(ci8.shape, dtype=bool)
        dup8[:, 1:] = ci8[:, 1:] == ci8[:, :-1]
        d8[dup8] = np.inf
        order = np.argsort(d8, axis=1, kind="stable")[:, :k]
        topk_idx[r0:r1] = np.take_along_axis(ci8, order, axis=1)

    lab = labels[topk_idx]               # [B, k] (int64)
    lab_kb = lab.reshape(k, B)           # faithful [B,k] -> [k,B] reshape
    outputs = lab_kb.sum(axis=0) // k
    out = np.zeros((B, L), dtype=np.float32)
    out[np.arange(B), outputs] = 1.0
    return out


# revision 63
# speedup vs baseline: 1.4736x; 1.2757x over previous
"""Distributed KNN (analogy-based estimation) kernel for 8 TRN2 NeuronCores.

Strategy (scan-then-refine):
  - Shard the train set (N=65536) across 8 cores (8192 rows each); replicate
    the 2048 queries.  All tensors fit in SBUF, so HBM traffic is just the
    ~3MB/core input load.  No collectives - the merge happens on the host.
  - Device scan: fp8(e4m3) DoubleRow matmuls (K=256 in one instruction)
    compute s = scale * (x_hat . t) into PSUM f32.  The true distance's norm
    term only perturbs candidate ranking by O(1) while top-of-65536 gaps are
    O(10), so the cross term alone selects candidate cells safely.
  - Evacuation is the bottleneck (every PSUM value must cross a compute
    engine at ~1 elem/lane/cycle), so it is split three ways: 1 of 4 PSUM
    tiles goes straight to VectorE (reduce_max over 32-candidate cells);
    the other 3 go ScalarE (fused relu(s - T), made row-comparable by host
    query normalization) -> GpSimd (ADD-fold of tile halves) -> VectorE
    (small sum-reduce), producing a 256-cell statistic vector per (row,
    core) that is DMA'd out raw (top-k selection on host beats on-device
    max/max_index by ~22us of DVE critical path).
  - Host: top-16 cells per (row, core) by statistic, expand to ~4k candidate
    indices/row as contiguous 16-row blocks, coarse f32 distance pass
    narrows to 8 finalists, exact float64 pass ranks them with the
    reference's tie-breaking, then the label gather / faithful [B,k]->[k,B]
    reshape / integer-mean / one-hot epilogue in exact integer arithmetic.
"""

from contextlib import ExitStack

import numpy as np
import ml_dtypes

import concourse.bass as bass
import concourse.mybir as mybir
import concourse.tile as tile
from concourse import bacc
from concourse.bass_utils import run_bass_kernel_spmd

N_CORES = 8
B = 2048          # queries
N_TRAIN = 65536   # train rows
F = 256           # features
NSHARD = N_TRAIN // N_CORES   # 8192 train rows per core

Q_TILE = 128
N_QT = B // Q_TILE            # 16 query tiles
CHUNK_N = 512                 # matmul free dim == one PSUM bank (fp32)
N_CHUNKS = NSHARD // CHUNK_N  # 16
TILE_W = 1024                 # psum tile width (2 banks, 2 chunks)
N_PT = NSHARD // TILE_W       # 8 psum tiles per (q-tile, core)
CELLS_PER_TILE = 32           # scan cells per psum tile
N_CELLS = N_PT * CELLS_PER_TILE    # 256 cells per row per core
# Cell c of psum tile m covers candidate columns
#   m*1024 + 16c + [0..15]  and  m*1024 + 512 + 16c + [0..15]
# (32 candidates per cell; the split pairing comes from the GpSimd fold).
TOPC_HALF = 8                 # cells reported per (row, core, cmax-half)
N_HALVES = 2
TOPC = TOPC_HALF * N_HALVES   # 16 cells reported per (row, core)
# Scan statistic: queries are L2-normalized on the host, so s = x_hat . t has
# per-candidate std ~1 and global top-3 values ~3.5+.  Cells are ranked by
# sum(relu(s - RELU_T)) (or relu(max - RELU_T) on the DVE-direct tiles) —
# any cell holding a global top-3 value scores far above typical cells.  The
# top-8 is taken independently over each 128-cell half so a global top-3
# cell would need >= 8 stronger cells in its own half to be lost.
RELU_T = 2.5

_BF16 = mybir.dt.bfloat16
_F32 = mybir.dt.float32
_U32 = mybir.dt.uint32


DIRECT_MOD = 4   # psum tiles with m % DIRECT_MOD == 0 go DVE-direct
PE_ONLY = False  # benchmark probe: skip all PSUM evacuation
FP8 = True       # fp8(e4m3) DoubleRow matmul scan (K=256 per MM) vs bf16
FP8_SCALE = 32.0  # pre-scale on normalized queries so fp8 stays in range
NO_L2 = False    # benchmark probe: skip the top-8 max/max_index stage
NO_POOL = False  # benchmark probe: DVE reduces ACT output (no GpSimd fold)


def _build(loop_reps=None):
    in_dt = mybir.dt.float8e4 if FP8 else _BF16
    nc = bacc.Bacc("TRN2", target_bir_lowering=False, debug=False)
    xT = nc.dram_tensor("xT", [F, B], in_dt, kind="ExternalInput")
    tT = nc.dram_tensor("tT", [F, NSHARD], in_dt, kind="ExternalInput")
    out_cm = nc.dram_tensor("cmax_out", [B, N_CELLS], _F32, kind="ExternalOutput")

    with tile.TileContext(nc) as tc, ExitStack() as ctx:
        const = ctx.enter_context(tc.tile_pool(name="const", bufs=1))
        psums = ctx.enter_context(tc.tile_pool(name="ps", bufs=4, space="PSUM"))
        cmaxp = ctx.enter_context(tc.tile_pool(name="cmax", bufs=2))
        stagep = ctx.enter_context(tc.tile_pool(name="stage", bufs=3))
        gpsp = ctx.enter_context(tc.tile_pool(name="gps", bufs=3))

        # Load order matters: the first PSUM wave needs xT q-tile 0 and tT
        # chunks 0..7, so stage those DMAs first.
        xT_sb = [[None] * N_QT for _ in range(2)]
        tT_sb = [[None] * N_CHUNKS for _ in range(2)]

        if FP8:
            # Bulk loads: one [128, 2*SIZE] tile per tensor holding both
            # 128-feature halves ((i, col) free layout); chunk operands are
            # strided [p, 2, w] views for DoubleRow.  Two 1MB DMAs stream
            # much faster than 32 64KB ones (shorter prologue).
            x_all = const.tile([128, 2 * B], mybir.dt.float8e4, name="x_all")
            t_all = const.tile([128, 2 * NSHARD], mybir.dt.float8e4,
                               name="t_all")

            def load_x(q):
                if q == 0:
                    for f in range(2):
                        nc.sync.dma_start(
                            x_all[:, f * B:(f + 1) * B], xT[f * 128:(f + 1) * 128, :]
                        )

            def load_t(c):
                if c == 0:
                    for f in range(2):
                        nc.sync.dma_start(
                            t_all[:, f * NSHARD:(f + 1) * NSHARD],
                            tT[f * 128:(f + 1) * 128, :],
                        )

            # [128, 2, w] strided views per q-tile / chunk for DoubleRow.
            x_dr = x_all[:].rearrange("p (i qw) -> p i qw", i=2)
            t_dr = t_all[:].rearrange("p (i cw) -> p i cw", i=2)
            xT_sb[0] = [
                x_dr[:, :, q * Q_TILE:(q + 1) * Q_TILE] for q in range(N_QT)
            ]
            tT_sb[0] = [
                t_dr[:, :, c * CHUNK_N:(c + 1) * CHUNK_N] for c in range(N_CHUNKS)
            ]
        else:
            def load_x(q):
                for f in range(2):
                    xs = const.tile([128, Q_TILE], _BF16, tag=f"xT{f}_{q}",
                                    name=f"xT{f}_{q}")
                    nc.sync.dma_start(
                        xs[:], xT[f * 128:(f + 1) * 128, q * Q_TILE:(q + 1) * Q_TILE]
                    )
                    xT_sb[f][q] = xs

            def load_t(c):
                for f in range(2):
                    ts_ = const.tile([128, CHUNK_N], _BF16, tag=f"tT{f}_{c}",
                                     name=f"tT{f}_{c}")
                    nc.sync.dma_start(
                        ts_[:],
                        tT[f * 128:(f + 1) * 128, c * CHUNK_N:(c + 1) * CHUNK_N]
                    )
                    tT_sb[f][c] = ts_

        load_x(0)
        for c in range(N_CHUNKS):
            load_t(c)
        for q in range(1, N_QT):
            load_x(q)

        neg_t = const.tile([128, 1], _F32, name="neg_t")
        nc.vector.memset(neg_t[:], -RELU_T * (FP8_SCALE if FP8 else 1.0))

        CPW = TILE_W // CHUNK_N  # chunks per psum tile
        TILES_PER_WAVE = 2       # tiles per accumulation wave
        WAVES = N_PT // TILES_PER_WAVE  # 4 waves per q-tile

        def compute():
            _compute(nc, tc, xT_sb, tT_sb, neg_t, cmaxp, psums, stagep,
                     gpsp, out_cm, CPW, TILES_PER_WAVE, WAVES)

        if loop_reps is not None:
            with tc.For_i(0, loop_reps, 1):
                compute()
        else:
            compute()
    nc.compile()
    return nc


def _compute(nc, tc, xT_sb, tT_sb, neg_t, cmaxp, psums, stagep, gpsp,
             out_cm, CPW, TILES_PER_WAVE, WAVES):
        cmaxes = {}

        def emit_wave(q, w, cmax):
                pss = [
                    psums.tile([128, TILE_W], _F32, tag="ps", name=f"ps_{q}_{w}_{j}")
                    for j in range(TILES_PER_WAVE)
                ]
                if FP8:
                    for j in range(TILES_PER_WAVE):
                        for hh in range(CPW):
                            c = (w * TILES_PER_WAVE + j) * CPW + hh
                            nc.tensor.matmul(
                                pss[j][:, hh * CHUNK_N:(hh + 1) * CHUNK_N],
                                xT_sb[0][q],
                                tT_sb[0][c],
                                start=True,
                                stop=True,
                                perf_mode=mybir.MatmulPerfMode.DoubleRow,
                            )
                else:
                    for f in range(2):  # contraction tiles of 128 features
                        for j in range(TILES_PER_WAVE):
                            for hh in range(CPW):
                                c = (w * TILES_PER_WAVE + j) * CPW + hh
                                nc.tensor.matmul(
                                    pss[j][:, hh * CHUNK_N:(hh + 1) * CHUNK_N],
                                    xT_sb[f][q][:],
                                    tT_sb[f][c][:],
                                    start=(f == 0),
                                    stop=(f == 1),
                                )
                for j in range(TILES_PER_WAVE):
                    m = w * TILES_PER_WAVE + j
                    cm_out = cmax[:, m * CELLS_PER_TILE:(m + 1) * CELLS_PER_TILE]
                    if PE_ONLY:
                        if m == 0:
                            nc.vector.memset(cmax[:], 0.0)
                        continue
                    if DIRECT_MOD > 0 and m % DIRECT_MOD == 0:
                        # DVE evacuates+reduces this tile straight from PSUM
                        # (max over 32 contiguous candidates per cell), then
                        # ScalarE maps it onto the relu(.-T) scale.
                        # Raw max out; the host subtracts RELU_T from these
                        # columns before selection (saves an ACT op here).
                        nc.vector.tensor_reduce(
                            out=cm_out,
                            in_=pss[j][:].rearrange("p (c e) -> p c e", e=32),
                            axis=mybir.AxisListType.X,
                            op=mybir.AluOpType.max,
                        )
                    else:
                        # Offload evacuation: ScalarE applies relu(s - T) on
                        # the way PSUM->SBUF, GpSimd ADD-folds the two
                        # 512-halves, DVE sum-reduces 16-wide cells.
                        st = stagep.tile([128, TILE_W], _BF16, tag="st",
                                         name=f"st_{q}_{m}")
                        nc.scalar.activation(
                            st[:], pss[j][:],
                            mybir.ActivationFunctionType.Relu,
                            bias=neg_t[:],
                        )
                        if NO_POOL:
                            nc.vector.tensor_reduce(
                                out=cm_out,
                                in_=st[:].rearrange(
                                    "p (two c e) -> p c two e", two=2, e=16
                                ),
                                axis=mybir.AxisListType.XY,
                                op=mybir.AluOpType.add,
                            )
                        else:
                            gp = gpsp.tile([128, TILE_W // 2], _BF16, tag="gp",
                                           name=f"gp_{q}_{m}")
                            nc.gpsimd.tensor_add(
                                gp[:], st[:, 0:TILE_W // 2],
                                st[:, TILE_W // 2:TILE_W]
                            )
                            nc.vector.tensor_reduce(
                                out=cm_out,
                                in_=gp[:].rearrange("p (c e) -> p c e", e=16),
                                axis=mybir.AxisListType.X,
                                op=mybir.AluOpType.add,
                            )
        for q in range(N_QT):
            cmax = cmaxp.tile([128, N_CELLS], _F32, name=f"cmax_{q}")
            for w in range(WAVES):
                emit_wave(q, w, cmax)
            qs = slice(q * Q_TILE, (q + 1) * Q_TILE)
            nc.sync.dma_start(out_cm[qs, :], cmax[:])


def _cells_to_blocks(cid):
    """Map per-(core,row) cell ids -> the two 16-row train blocks each covers.

    cid: int64 array of cell ids in [0, N_CELLS), AFTER the half-offset fix.
    Returns (blk0, blk1) within-shard block indices (block = 16 train rows).
    DVE-direct tiles (m % DIRECT_MOD == 0) use 32 contiguous candidates;
    offloaded tiles pair candidates {16c, 16c+512} (the GpSimd fold).
    """
    m = cid // CELLS_PER_TILE
    c = cid % CELLS_PER_TILE
    direct = (m % DIRECT_MOD == 0) if DIRECT_MOD > 0 else np.zeros_like(m, bool)
    blk0 = m * (TILE_W // 16) + np.where(direct, 2 * c, c)
    blk1 = blk0 + np.where(direct, 1, TILE_W // 32)
    return blk0, blk1


def _host_adjust(cm):
    """Direct tiles report raw cell max; put them on the relu(.-T) scale."""
    shift = RELU_T * (FP8_SCALE if FP8 else 1.0)
    m = np.arange(N_CELLS) // CELLS_PER_TILE
    direct = (m % DIRECT_MOD == 0) if DIRECT_MOD > 0 else np.zeros_like(m, bool)
    cm[..., direct] -= shift
    return cm


_CACHE = {}


def _run_device(x_input, train_inputs, trace=False, **kw):
    if "nc" not in _CACHE:
        _CACHE["nc"] = _build()
    nc = _CACHE["nc"]
    x = np.asarray(x_input, np.float32)
    # Row-normalize queries so the global RELU_T threshold is calibrated.
    xh = x / (np.linalg.norm(x, axis=1, keepdims=True) + 1e-30)
    if FP8:
        in_np_dt = ml_dtypes.float8_e4m3
        xh = xh * FP8_SCALE
    else:
        in_np_dt = ml_dtypes.bfloat16
    xT = np.ascontiguousarray(xh.T).astype(in_np_dt)
    in_maps = []
    for s in range(N_CORES):
        shard = np.asarray(train_inputs[s * NSHARD:(s + 1) * NSHARD], np.float32)
        tTs = np.ascontiguousarray(shard.T).astype(in_np_dt)
        in_maps.append({"xT": xT, "tT": tTs})
    return run_bass_kernel_spmd(
        nc, in_maps, core_ids=list(range(N_CORES)), trace=trace, **kw
    )


def kernel(x_input, train_inputs, features, train_labels, num_k, num_labels):
    x = np.asarray(x_input, dtype=np.float32)
    train = np.asarray(train_inputs, dtype=np.float32)
    feats = np.asarray(features, dtype=np.float32)
    labels = np.asarray(train_labels)
    k = int(num_k)
    L = int(num_labels)

    res = _run_device(x, train)
    cm = np.stack(
        [np.asarray(res.results[s]["cmax_out"]) for s in range(N_CORES)], axis=0
    )  # [cores, B, N_CELLS] f32 cell statistics
    cm = _host_adjust(cm)

    # Host-side selection: top-TOPC cells per (core, row) by statistic.
    flat = cm.reshape(-1, N_CELLS)
    part = np.argpartition(-flat, TOPC - 1, axis=1)[:, :TOPC]
    cid = part.reshape(N_CORES, B, TOPC).astype(np.int64)

    # Expand top cells to candidate BLOCKS of 16 contiguous train rows.
    blk0, blk1 = _cells_to_blocks(cid)
    blk = np.stack([blk0, blk1], axis=-1)             # [cores, B, TOPC, 2]
    blk = blk + (np.arange(N_CORES, dtype=np.int64) * (NSHARD // 16))[
        :, None, None, None
    ]
    blk = blk.transpose(1, 0, 2, 3).reshape(B, -1)    # [B, cores*TOPC*2=256]
    blk = np.sort(blk, axis=1)
    NBLK = blk.shape[1]
    dupb = np.zeros(blk.shape, dtype=bool)
    dupb[:, 1:] = blk[:, 1:] == blk[:, :-1]

    # Refinement: coarse f32 pass narrows ~4k candidates/row to 8, then an
    # exact float64 pass ranks those with the reference's tie-breaking.
    w = feats[None, :] * train
    right32 = np.einsum("nf,nf->n", w, w, dtype=np.float32)
    left32 = np.einsum("bf,bf->b", x, x, dtype=np.float32)
    w64 = w.astype(np.float64)
    x64 = x.astype(np.float64)
    left64 = np.einsum("bf,bf->b", x64, x64)

    train_blocks = train.reshape(N_TRAIN // 16, 16 * F)
    NARROW = 8
    topk_idx = np.empty((B, k), dtype=np.int64)
    CH = 128
    gbuf = np.empty((CH * NBLK, 16 * F), dtype=np.float32)
    for r0 in range(0, B, CH):
        r1 = min(B, r0 + CH)
        bi = blk[r0:r1]                                # [rows, NBLK]
        ci = (bi[:, :, None] * 16 + np.arange(16)).reshape(r1 - r0, -1)
        np.take(train_blocks, bi.ravel(), axis=0, out=gbuf)
        tcand = gbuf.reshape(r1 - r0, NBLK * 16, F)    # [rows, nc, F]
        cross = np.matmul(tcand, x[r0:r1][:, :, None])[..., 0]
        d32 = np.sqrt(left32[r0:r1, None] + right32[ci]) - 2.0 * cross
        d32.reshape(r1 - r0, NBLK, 16)[dupb[r0:r1]] = np.inf
        part = np.argpartition(d32, NARROW, axis=1)[:, :NARROW]
        ci8 = np.take_along_axis(ci, part, axis=1)     # [rows, 8] distinct
        ci8.sort(axis=1)
        # exact f64 distances for the 8 finalists
        t8 = train[ci8].astype(np.float64)
        cross8 = np.matmul(t8, x64[r0:r1][:, :, None])[..., 0]
        w8 = w64[ci8]
        r8 = np.einsum("bkf,bkf->bk", w8, w8)
        d8 = np.sqrt(left64[r0:r1, None] + r8) - 2.0 * cross8
        dup8 = np.zeros(ci8.shape, dtype=bool)
        dup8[:, 1:] = ci8[:, 1:] == ci8[:, :-1]
        d8[dup8] = np.inf
        order = np.argsort(d8, axis=1, kind="stable")[:, :k]
        topk_idx[r0:r1] = np.take_along_axis(ci8, order, axis=1)

    lab = labels[topk_idx]               # [B, k] (int64)
    lab_kb = lab.reshape(k, B)           # faithful [B,k] -> [k,B] reshape
    outputs = lab_kb.sum(axis=0) // k
    out = np.zeros((B, L), dtype=np.float32)
    out[np.arange(B), outputs] = 1.0
    return out
